# revision 5
# baseline (speedup 1.0000x reference)
"""ODE-RNN encoder (GRU-ODE scan) Trainium2 Bass kernel, v2.

Strategy (data-parallel over trajectories):
  - 4096 trajectories sharded 512/core over 8 NeuronCores; weights
    replicated; T=128 time scan runs locally per core. Host gathers the
    per-core z0 outputs.
  - Feature-on-partition, batch-on-free-dim layout. Each core's 512-batch
    splits into 2 chunks of 256. Chunk 1 is emitted one FULL time-step
    behind chunk 0, interleaved op-by-op, so (a) every engine queue
    alternates ops of two independent dependency chains and (b) the two
    chunks' same-weight matmuls are adjacent on the PE queue, letting the
    second matmul of each pair skip its LDWEIGHTS (weights stay in the PE
    array). Weights are bf16 (2x faster weight loads); moving operands
    are mode-selectable (f32r / bf16).
  - The mask blend m*(...) is folded into the u-gate sigmoid:
    g = m*(1-u) = sigmoid(-pre_u + C*(m-1)) with C=30, realized by two
    extra accumulating matmuls (C*[I;I] @ mask and a K=1 -C row), which
    removes the mask-duplication DMAs and the g=m*v vector multiply.
  - Blend restructured as d = [ns_y; |ns_s|] - S (one full-width DVE sub,
    replacing the old negI matmul + split subs), then d *= g, S' = S + d.
  - tanh(u-bank|r-bank) and sigmoid(g-bank|r-bank) each run as one
    512-column activation over a shared PSUM bank.
  - The ODE correction to the gate-1 layers is algebraically folded into
    host-precomputed dt*(ode_w2 @ W_g1[:64]) weights so gates start from
    the OLD state in parallel with the ODE evaluation.

kernel(**inputs) takes the full unsharded numpy inputs and returns
(z0_mu, z0_std), each (1, 4096, 64) float32.
"""

import sys

import numpy as np

N_TRAJ = 4096
T = 128
LAT = 64
NDATA = 64
INP = 2 * NDATA
NGRU = 100
NODE = 100
TZ = 100
NCORES = 8
B = N_TRAJ // NCORES          # 512 per core
CH = 2                        # chunks per core
BC = B // CH                  # 256 batch per chunk
CMASK = 30.0                  # mask fold constant: sigmoid(-30) ~ 9e-14

# Moving-operand dtype mode:
#   "bf16"  : everything bf16 (fastest DVE, lowest accuracy)
#   "cast"  : state fp32 (exact ODE/blend accumulate) + per-step bf16 copy
#             of the state for the matmul operands
RHS_MODE = "bf16"

_cache = {}


def _build(dts, rhs_mode):
    import concourse.bass as bass
    import concourse.tile as tile
    from concourse import bacc, mybir

    uniq = list(dict.fromkeys(dts))
    dt_idx = [uniq.index(d) for d in dts]
    n_dt = len(uniq)

    f32 = mybir.dt.float32
    f32r = mybir.dt.float32r
    bf16 = mybir.dt.bfloat16
    ACT = mybir.ActivationFunctionType
    ALU = mybir.AluOpType

    cast = rhs_mode == "cast"
    mv = bf16                                     # matmul moving dtype
    st = f32 if cast else bf16                    # state dtype

    nc = bacc.Bacc("TRN2", target_bir_lowering=False, debug=False,
                   num_devices=NCORES)

    # ---- DRAM I/O ----
    xT_d = nc.dram_tensor("xT", [T, INP, B], mv, kind="ExternalInput")
    wug1a_d = nc.dram_tensor("wug1a", [2 * LAT, NGRU], bf16, kind="ExternalInput")
    wug1b_d = nc.dram_tensor("wug1b", [INP, NGRU], bf16, kind="ExternalInput")
    wrg1a_d = nc.dram_tensor("wrg1a", [2 * LAT, NGRU], bf16, kind="ExternalInput")
    wrg1b_d = nc.dram_tensor("wrg1b", [INP, NGRU], bf16, kind="ExternalInput")
    wns1a_d = nc.dram_tensor("wns1a", [2 * LAT, NGRU], bf16, kind="ExternalInput")
    wns1b_d = nc.dram_tensor("wns1b", [INP, NGRU], bf16, kind="ExternalInput")
    wug2_d = nc.dram_tensor("wug2nd", [NGRU, 2 * LAT], bf16, kind="ExternalInput")
    wrg2_d = nc.dram_tensor("wrg2d", [NGRU, 2 * LAT], bf16, kind="ExternalInput")
    wns2_d = nc.dram_tensor("wns2", [NGRU, 2 * LAT], bf16, kind="ExternalInput")
    wode1_d = nc.dram_tensor("wode1", [LAT, NODE], bf16, kind="ExternalInput")
    wode2_d = nc.dram_tensor("wode2", [NODE, LAT], bf16, kind="ExternalInput")
    wfug_d = nc.dram_tensor("wfuse_ug", [n_dt, NODE, NGRU], bf16, kind="ExternalInput")
    wfrg_d = nc.dram_tensor("wfuse_rg", [n_dt, NODE, NGRU], bf16, kind="ExternalInput")
    cii_d = nc.dram_tensor("cii", [LAT, 2 * LAT], bf16, kind="ExternalInput")
    negc_d = nc.dram_tensor("negc", [1, 2 * LAT], bf16, kind="ExternalInput")
    ones_d = nc.dram_tensor("ones1", [1, BC], mv, kind="ExternalInput")
    wtz1_d = nc.dram_tensor("wtz1", [2 * LAT, TZ], bf16, kind="ExternalInput")
    wtz2_d = nc.dram_tensor("wtz2", [TZ, 2 * LAT], bf16, kind="ExternalInput")
    zeros_d = nc.dram_tensor("zeros0", [2 * LAT, B], mv, kind="ExternalInput")
    zout_d = nc.dram_tensor("zout", [2 * LAT, B], f32, kind="ExternalOutput")

    def c32(ap):
        return ap.bitcast(f32)

    with tile.TileContext(nc) as tc:
        with (
            tc.tile_pool(name="const", bufs=1) as cpool,
            tc.tile_pool(name="state", bufs=1) as spool,
            tc.tile_pool(name="xin", bufs=3) as xpool,
            tc.tile_pool(name="tmp0", bufs=2) as tpool0,
            tc.tile_pool(name="tmp1", bufs=2) as tpool1,
            tc.tile_pool(name="psA0", bufs=1, space="PSUM") as psA0,
            tc.tile_pool(name="psB0", bufs=1, space="PSUM") as psB0,
            tc.tile_pool(name="g1p0", bufs=1, space="PSUM") as g1p0,
            tc.tile_pool(name="g2p0", bufs=1, space="PSUM") as g2p0,
            tc.tile_pool(name="psA1", bufs=1, space="PSUM") as psA1,
            tc.tile_pool(name="psB1", bufs=1, space="PSUM") as psB1,
            tc.tile_pool(name="g1p1", bufs=1, space="PSUM") as g1p1,
            tc.tile_pool(name="g2p1", bufs=1, space="PSUM") as g2p1,
        ):
            tpool = [tpool0, tpool1]
            psA = [psA0, psA1]
            psB = [psB0, psB1]
            g1p = [g1p0, g1p1]
            g2p = [g2p0, g2p1]

            # ---- load constants ----
            def cload(shape, src_ap, tag, dt_=bf16):
                t = cpool.tile(shape, dt_, tag=tag, name=tag)
                nc.sync.dma_start(t[:, :], src_ap)
                return t

            wug1a = cload([2 * LAT, NGRU], wug1a_d[:, :], "wug1a")
            wug1b = cload([INP, NGRU], wug1b_d[:, :], "wug1b")
            wrg1a = cload([2 * LAT, NGRU], wrg1a_d[:, :], "wrg1a")
            wrg1b = cload([INP, NGRU], wrg1b_d[:, :], "wrg1b")
            wns1a = cload([2 * LAT, NGRU], wns1a_d[:, :], "wns1a")
            wns1b = cload([INP, NGRU], wns1b_d[:, :], "wns1b")
            wug2 = cload([NGRU, 2 * LAT], wug2_d[:, :], "wug2")
            wrg2 = cload([NGRU, 2 * LAT], wrg2_d[:, :], "wrg2")
            wns2 = cload([NGRU, 2 * LAT], wns2_d[:, :], "wns2")
            wode1 = cload([LAT, NODE], wode1_d[:, :], "wode1")
            wode2 = cload([NODE, LAT], wode2_d[:, :], "wode2")
            wtz1 = cload([2 * LAT, TZ], wtz1_d[:, :], "wtz1")
            wtz2 = cload([TZ, 2 * LAT], wtz2_d[:, :], "wtz2")
            wfug = [cload([NODE, NGRU], wfug_d[i], f"wfug{i}") for i in range(n_dt)]
            wfrg = [cload([NODE, NGRU], wfrg_d[i], f"wfrg{i}") for i in range(n_dt)]
            # C*[I I] lives on partitions 64:128 so its base partition
            # matches the mask half of xt (matmul reads both from the same
            # SBUF partitions).
            cii = cpool.tile([INP, 2 * LAT], bf16, tag="cii", name="cii")
            nc.sync.dma_start(cii[NDATA:INP, :], cii_d[:, :])
            negc = cload([1, 2 * LAT], negc_d[:, :], "negc")
            ones = cpool.tile([1, BC], mv, tag="ones", name="ones")
            nc.sync.dma_start(ones[:, :], ones_d[:, :])

            # ---- state tiles (ping-pong per chunk) ----
            S = [[spool.tile([2 * LAT, BC], mv, tag=f"s{c}_{p}",
                             name=f"s{c}_{p}")
                  for p in range(2)] for c in range(CH)]
            for c in range(CH):
                nc.sync.dma_start(S[c][0][:, :],
                                  zeros_d[:, c * BC:(c + 1) * BC])

            # ---- the scan ----
            # Stage functions emit ops for chunk c at step t and return a
            # list of (matmul_handle, weight_key) for LD pairing.
            def new_ctx(c, t):
                return dict(cs=slice(c * BC, (c + 1) * BC),
                            Sc=S[c][t % 2], Sn=S[c][(t + 1) % 2],
                            tp=tpool[c], t=t)

            xts = {}

            def s_x(c, d, t):
                # prefetch next step's x (chunk0 slot only)
                if c == 0 and t + 1 < T:
                    xt = xpool.tile([INP, B], mv, tag="xt",
                                    name=f"xt{(t + 1) % 3}")
                    nc.sync.dma_start(xt[:, :], xT_d[t + 1])
                    xts[t + 1] = xt
                return []

            def s_ode1(c, d, t):
                d['ps_oh'] = psA[c].tile([NODE, BC], f32, tag="psA",
                                         name=f"oh{c}")
                h = nc.tensor.matmul(d['ps_oh'][:, :], wode1[:, :],
                                     d['Sc'][0:LAT, :], start=True, stop=True)
                return [(h, "wode1")]

            def s_tanh_o(c, d, t):
                d['h_ode'] = d['tp'].tile([NODE, BC], mv, tag="h_ode",
                                          name=f"ho{c}")
                nc.scalar.activation(d['h_ode'][:, :], d['ps_oh'][:, :],
                                     ACT.Tanh)
                return []

            def s_rg1x(c, d, t):
                d['g1'] = g1p[c].tile([NGRU, 2 * BC], f32, tag="g1",
                                      name=f"g1_{c}")
                h = nc.tensor.matmul(d['g1'][:, BC:2 * BC], wrg1b[:, :],
                                     xts[t][:, d['cs']], start=True,
                                     stop=False)
                return [(h, "wrg1b")]

            def s_rg1s(c, d, t):
                h = nc.tensor.matmul(d['g1'][:, BC:2 * BC], wrg1a[:, :],
                                     d['Sc'][:, :], start=False, stop=False)
                return [(h, "wrg1a")]

            def s_rg1c(c, d, t):
                h = nc.tensor.matmul(d['g1'][:, BC:2 * BC],
                                     wfrg[dt_idx[t]][:, :],
                                     d['h_ode'][:, :], start=False, stop=True)
                return [(h, ("wfrg", dt_idx[t]))]

            def s_ug1x(c, d, t):
                h = nc.tensor.matmul(d['g1'][:, 0:BC], wug1b[:, :],
                                     xts[t][:, d['cs']], start=True,
                                     stop=False)
                return [(h, "wug1b")]

            def s_ug1s(c, d, t):
                h = nc.tensor.matmul(d['g1'][:, 0:BC], wug1a[:, :],
                                     d['Sc'][:, :], start=False, stop=False)
                return [(h, "wug1a")]

            def s_ug1c(c, d, t):
                h = nc.tensor.matmul(d['g1'][:, 0:BC],
                                     wfug[dt_idx[t]][:, :],
                                     d['h_ode'][:, :], start=False, stop=True)
                return [(h, ("wfug", dt_idx[t]))]

            def s_tanh_ur(c, d, t):
                d['h_g'] = d['tp'].tile([NGRU, 2 * BC], mv, tag="h_g",
                                        name=f"hg{c}")
                nc.scalar.activation(d['h_g'][:, :], d['g1'][:, :], ACT.Tanh)
                return []

            def s_ode2(c, d, t):
                d['ps_yo'] = psB[c].tile([LAT, BC], f32, tag="psB",
                                         name=f"yo{c}")
                h = nc.tensor.matmul(d['ps_yo'][:, :], wode2[:, :],
                                     d['h_ode'][:, :], start=True, stop=True)
                return [(h, "wode2")]

            def s_yode(c, d, t):
                nc.vector.scalar_tensor_tensor(
                    d['Sc'][0:LAT, :], d['ps_yo'][:, :], float(dts[t]),
                    c32(d['Sc'][0:LAT, :]) if mv is f32r else d['Sc'][0:LAT, :],
                    op0=ALU.mult, op1=ALU.add)
                return []

            def s_rg2(c, d, t):
                d['g2'] = g2p[c].tile([2 * LAT, 2 * BC], f32, tag="g2",
                                      name=f"g2_{c}")
                h = nc.tensor.matmul(d['g2'][:, BC:2 * BC], wrg2[:, :],
                                     d['h_g'][:, BC:2 * BC],
                                     start=True, stop=True)
                return [(h, "wrg2")]

            def s_ug2(c, d, t):
                h = nc.tensor.matmul(d['g2'][:, 0:BC], wug2[:, :],
                                     d['h_g'][:, 0:BC], start=True, stop=False)
                return [(h, "wug2")]

            def s_maskc(c, d, t):
                h = nc.tensor.matmul(d['g2'][:, 0:BC], cii[NDATA:INP, :],
                                     xts[t][NDATA:INP, d['cs']],
                                     start=False, stop=False)
                return [(h, "cii")]

            def s_negc(c, d, t):
                h = nc.tensor.matmul(d['g2'][:, 0:BC], negc[:, :],
                                     ones[:, :], start=False, stop=True)
                return [(h, "negc")]

            def s_sig(c, d, t):
                d['vr'] = d['tp'].tile([2 * LAT, 2 * BC], mv, tag="vr",
                                       name=f"vr{c}")
                nc.scalar.activation(d['vr'][:, :], d['g2'][:, :], ACT.Sigmoid)
                return []

            def s_ryc(c, d, t):
                d['ryc'] = d['tp'].tile([2 * LAT, BC], mv, tag="ryc",
                                        name=f"ryc{c}")
                nc.vector.tensor_mul(d['ryc'][:, :], d['vr'][:, BC:2 * BC],
                                     d['Sc'][:, :])
                return []

            def s_ns1x(c, d, t):
                d['n1'] = psA[c].tile([NGRU, BC], f32, tag="psA",
                                      name=f"n1_{c}")
                h = nc.tensor.matmul(d['n1'][:, :], wns1b[:, :],
                                     xts[t][:, d['cs']], start=True,
                                     stop=False)
                return [(h, "wns1b")]

            def s_ns1r(c, d, t):
                h = nc.tensor.matmul(d['n1'][:, :], wns1a[:, :],
                                     d['ryc'][:, :], start=False, stop=True)
                return [(h, "wns1a")]

            def s_tanh_n(c, d, t):
                d['h_n'] = d['tp'].tile([NGRU, BC], mv, tag="h_n",
                                        name=f"hn{c}")
                nc.scalar.activation(d['h_n'][:, :], d['n1'][:, :], ACT.Tanh)
                return []

            def s_ns2(c, d, t):
                d['n2'] = psB[c].tile([2 * LAT, BC], f32, tag="psB",
                                      name=f"n2_{c}")
                h = nc.tensor.matmul(d['n2'][:, :], wns2[:, :],
                                     d['h_n'][:, :], start=True, stop=True)
                return [(h, "wns2")]

            def s_absb(c, d, t):
                n2 = d['n2']
                nc.scalar.activation(n2[LAT:2 * LAT, :], n2[LAT:2 * LAT, :],
                                     ACT.Abs)
                return []

            def s_dsub(c, d, t):
                d['dd'] = d['tp'].tile([2 * LAT, BC], mv, tag="dd",
                                       name=f"dd{c}")
                nc.vector.tensor_sub(d['dd'][:, :], d['n2'][:, :],
                                     d['Sc'][:, :])
                return []

            def s_gtq(c, d, t):
                nc.vector.tensor_mul(d['dd'][:, :], d['vr'][:, 0:BC],
                                     d['dd'][:, :])
                return []

            def s_add(c, d, t):
                nc.vector.tensor_add(d['Sn'][:, :], d['Sc'][:, :],
                                     d['dd'][:, :])
                return []

            stages = [s_x, s_ode1, s_tanh_o, s_rg1x, s_rg1s, s_rg1c,
                      s_ug1x, s_ug1s, s_ug1c, s_tanh_ur, s_ode2, s_yode,
                      s_rg2, s_ug2, s_maskc, s_negc, s_sig, s_ryc,
                      s_ns1x, s_ns1r, s_tanh_n, s_ns2, s_absb, s_dsub,
                      s_gtq, s_add]
            NS = len(stages)

            # preload x(0)
            xt0 = xpool.tile([INP, B], mv, tag="xt", name="xt0")
            nc.sync.dma_start(xt0[:, :], xT_d[0])
            xts[0] = xt0

            ctx = [None, None]
            for n in range(NS * (T + 1)):
                t0, k = divmod(n, NS)
                mm0 = []
                if t0 < T:
                    if k == 0:
                        ctx[0] = new_ctx(0, t0)
                    mm0 = stages[k](0, ctx[0], t0)
                t1 = t0 - 1
                if t1 >= 0:
                    if k == 0:
                        ctx[1] = new_ctx(1, t1)
                    mm1 = stages[k](1, ctx[1], t1)
                    for (h0, k0), (h1, k1) in zip(mm0, mm1):
                        if k0 == k1:
                            h1.ins.ldweights = False
                    if k == NS - 1:
                        xts.pop(t1, None)

            # ---- final transform z0 = mlp2([y; s]) ----
            for c in range(CH):
                cs = slice(c * BC, (c + 1) * BC)
                Sf = S[c][T % 2]
                pt1 = psA[c].tile([TZ, BC], f32, tag="psA")
                nc.tensor.matmul(pt1[:, :], wtz1[:, :], Sf[:, :],
                                 start=True, stop=True)
                h_t = tpool[c].tile([TZ, BC], mv, tag="h_t")
                nc.scalar.activation(h_t[:, :], pt1[:, :], ACT.Tanh)
                pt2 = psB[c].tile([2 * LAT, BC], f32, tag="psB")
                nc.tensor.matmul(pt2[:, :], wtz2[:, :], h_t[:, :],
                                 start=True, stop=True)
                zo = tpool[c].tile([2 * LAT, BC], f32, tag="zo")
                nc.scalar.activation(zo[0:LAT, :], pt2[0:LAT, :], ACT.Copy)
                nc.scalar.activation(zo[LAT:2 * LAT, :], pt2[LAT:2 * LAT, :],
                                     ACT.Abs)
                nc.sync.dma_start(zout_d[:, cs], zo[:, :])

    nc.compile()
    return nc


def _prep(inputs, rhs_mode):
    import ml_dtypes
    bf = ml_dtypes.bfloat16

    g = lambda k: np.ascontiguousarray(np.asarray(inputs[k], dtype=np.float32))
    data = g("data")
    tps = g("tps")
    W = {k: g(k) for k in (
        "ug_w1", "ug_b1", "ug_w2", "ug_b2", "rg_w1", "rg_b1", "rg_w2", "rg_b2",
        "ns_w1", "ns_b1", "ns_w2", "ns_b2", "ode_w1", "ode_b1", "ode_w2",
        "ode_b2", "tz_w1", "tz_b1", "tz_w2", "tz_b2")}

    for k in W:
        if "_b" in k:
            assert float(np.abs(W[k]).max()) == 0.0, \
                f"nonzero bias {k} unsupported by this kernel"

    rev = tps[::-1]
    dts = np.concatenate([np.full((1,), -0.01, np.float32),
                          rev[1:] - rev[:-1]]).astype(np.float32)
    dts = tuple(float(d) for d in dts.tolist())

    # time-reverse + transpose: [T, INP, N_TRAJ], contiguous
    xT_full = np.ascontiguousarray(data[:, ::-1, :].transpose(1, 2, 0))

    mv_np = np.float32 if rhs_mode != "bf16" else bf

    uniq = list(dict.fromkeys(dts))
    eye = np.eye(LAT, dtype=np.float32)
    common = {
        "wfuse_ug": np.stack([np.float32(d) * (W["ode_w2"] @ W["ug_w1"][:LAT])
                              for d in uniq]),
        "wfuse_rg": np.stack([np.float32(d) * (W["ode_w2"] @ W["rg_w1"][:LAT])
                              for d in uniq]),
        "wug1a": W["ug_w1"][:2 * LAT],
        "wug1b": W["ug_w1"][2 * LAT:],
        "wrg1a": W["rg_w1"][:2 * LAT],
        "wrg1b": W["rg_w1"][2 * LAT:],
        "wns1a": W["ns_w1"][:2 * LAT],
        "wns1b": W["ns_w1"][2 * LAT:],
        "wug2nd": -np.concatenate([W["ug_w2"], W["ug_w2"]], axis=1),
        "wrg2d": np.concatenate([W["rg_w2"], W["rg_w2"]], axis=1),
        "wns2": W["ns_w2"],
        "wode1": W["ode_w1"],
        "wode2": W["ode_w2"],
        "cii": CMASK * np.concatenate([eye, eye], axis=1),
        "negc": np.full((1, 2 * LAT), -CMASK, np.float32),
        "wtz1": W["tz_w1"],
        "wtz2": W["tz_w2"],
    }
    common = {k: np.ascontiguousarray(v.astype(bf))
              for k, v in common.items()}
    common["ones1"] = np.ones((1, BC), mv_np)
    common["zeros0"] = np.zeros((2 * LAT, B), mv_np)

    in_maps = []
    for c in range(NCORES):
        m = dict(common)
        m["xT"] = np.ascontiguousarray(
            xT_full[:, :, c * B:(c + 1) * B].astype(mv_np))
        in_maps.append(m)
    return in_maps, dts


def _ensure_ntff_hook():
    """run_bass_kernel_spmd(trace=True) under axon imports
    antenv.axon_hooks, which is absent in this image. Install a stub so a
    BASS_TRACE=1 environment cannot crash the run."""
    import types as _types
    if "antenv.axon_hooks" in sys.modules:
        return
    hook = None
    try:
        from trn_agent_boot.trn_boot import _ntff_profile_via_ctypes
        hook = _ntff_profile_via_ctypes("/opt/axon/libaxon_pjrt.so")
    except Exception:
        hook = None
    try:
        import antenv
        mod = _types.ModuleType("antenv.axon_hooks")
        mod.get_axon_ntff_profile_hook = lambda: hook
        mod.set_axon_ntff_profile_hook = lambda h: None
        sys.modules["antenv.axon_hooks"] = mod
        antenv.axon_hooks = mod
    except Exception:
        pass


def _run(inputs, trace=False, trace_kwargs=None):
    _ensure_ntff_hook()
    from concourse.bass_utils import run_bass_kernel_spmd

    in_maps, dts = _prep(inputs, RHS_MODE)
    key = (dts, RHS_MODE)
    if key not in _cache:
        _cache[key] = _build(dts, RHS_MODE)
    nc = _cache[key]

    res = run_bass_kernel_spmd(nc, in_maps, list(range(NCORES)),
                               trace=trace, **(trace_kwargs or {}))
    mu = np.empty((N_TRAJ, LAT), np.float32)
    std = np.empty((N_TRAJ, LAT), np.float32)
    for c in range(NCORES):
        z = res.results[c]["zout"]
        mu[c * B:(c + 1) * B] = z[0:LAT].T
        std[c * B:(c + 1) * B] = z[LAT:2 * LAT].T
    return (mu[None], std[None]), res


def kernel(**inputs):
    out, _ = _run(inputs, trace=False)
    return out
